# revision 25
# baseline (speedup 1.0000x reference)
"""Trainium2 Bass kernel for fused MultiHeadAttention + residual + LayerNorm.

Problem: B=2, L=S=2048, D=768, H=12 heads of dim 64, attention with key-padding
mask, output projection + bias, residual with q, LayerNorm(gamma, beta).

Sharding over 8 NeuronCores: data-parallel over batch (2 groups of 4 cores) x
tensor-parallel over heads (3 heads per core).

V2: all matmul operands in bf16 (f32 PSUM accumulation), l-block-sequential
attention with row-group-paired score matmuls, exp split between ScalarE
(true exp, wide [128,1024] activations) and DVE (one-op Schraudolph fast-exp
written as int16 and bitcast to bf16), softmax denominator via an extra ones
column on V, normalization via reciprocal_approx_fast + gpsimd broadcast,
K=128 head-stacked output projection, bf16 ReduceScatter in 8 chunks each
followed immediately by its LayerNorm (no ScalarE table switches: rstd via a
DVE Newton rsqrt).
"""

import sys

sys.path.insert(0, "/opt/trn_rl_repo")

import ml_dtypes
import numpy as np

import concourse.bass as bass
import concourse.tile as tile
from concourse import bacc, mybir
from concourse.bass_utils import run_bass_kernel_spmd

F32 = mybir.dt.float32
BF16 = mybir.dt.bfloat16
I16 = mybir.dt.int16
F8 = mybir.dt.float8e4
I32 = mybir.dt.int32

D = 768
HD = 64
HPC = 3  # heads per core
HCOLS = HPC * HD  # 192
B, L, S = 2, 2048, 2048
NCORES = 8
GROUPS = [[0, 1, 2, 3], [4, 5, 6, 7]]
KCH = D // 128  # 6 contraction chunks for projections
SCH = S // 128  # 16 s-chunks
LB = 512  # l-block width
NLB = L // LB  # 4
NCHUNK = 4  # ReduceScatter chunks (512 rows each)
CROWS = L // NCHUNK  # 512
ORows = CROWS // 4  # 128 rows per core per chunk
LN_EPS = 1e-5
MASK_NEG = -1000000.0

# Schraudolph fast-exp into bf16 bits: bits = (s + b_p) * FE_A with
# FE_A = 0.125 * 128/ln2 and b_p = 8*mask_bias + FE_B/FE_A.
FE_A = 0.125 * (128.0 / np.log(2.0))  # 23.0831...
FE_B = 127.0 * 128.0 - 7.42 + 0.5  # bias - Schraudolph C + trunc hedge
FE_B_OVER_A = FE_B / FE_A

# exp engine split knobs
WIDE_SCALAR = (True, True, False, True, True, False, True, False)  # 5/3 per 8
NARROW_SCALAR = (True, False, True, False, True, False, True, False)

_CACHE: dict = {}


def _build():
    nc = bacc.Bacc("TRN2", target_bir_lowering=False, debug=False, num_devices=NCORES)

    qT = nc.dram_tensor("qT", [D, L], BF16, kind="ExternalInput").ap()
    kT = nc.dram_tensor("kT", [D, S], BF16, kind="ExternalInput").ap()
    vT = nc.dram_tensor("vT", [D, S], BF16, kind="ExternalInput").ap()
    wqT = nc.dram_tensor("wqT", [D, HCOLS], BF16, kind="ExternalInput").ap()
    wkT = nc.dram_tensor("wkT", [D, HCOLS], BF16, kind="ExternalInput").ap()
    wvT = nc.dram_tensor("wvT", [D, HCOLS], BF16, kind="ExternalInput").ap()
    wT01 = nc.dram_tensor("wT01", [128, D], BF16, kind="ExternalInput").ap()
    wT2 = nc.dram_tensor("wT2", [64, D], BF16, kind="ExternalInput").ap()
    qres = nc.dram_tensor("qres", [NCHUNK, ORows, D], BF16, kind="ExternalInput").ap()
    maskT = nc.dram_tensor("maskT", [128, SCH], I32, kind="ExternalInput").ap()
    bias1 = nc.dram_tensor("bias1", [1, D], F32, kind="ExternalInput").ap()
    gamma1 = nc.dram_tensor("gamma1", [1, D], F32, kind="ExternalInput").ap()
    beta1 = nc.dram_tensor("beta1", [1, D], F32, kind="ExternalInput").ap()
    out = nc.dram_tensor("out", [NCHUNK, ORows, D], F32, kind="ExternalOutput").ap()

    AL = mybir.AluOpType
    ACT = mybir.ActivationFunctionType

    with tile.TileContext(nc, num_cores=NCORES) as tc:
        with (
            tc.tile_pool(name="persist", bufs=1) as pp,
            tc.tile_pool(name="dram", bufs=1, space="DRAM") as dram,
        ):
            QT1 = pp.tile([128, L], BF16)  # h0 feats on 0:64, h1 on 64:128
            QT2 = pp.tile([128, L], BF16)  # h2 feats, duplicated 64:128
            KT1 = pp.tile([128, S], BF16)
            KT2 = pp.tile([128, S], BF16)
            V_sb = pp.tile([128, SCH, HPC, 128], BF16)  # col 0 = ones (denom), 64:128 = V
            OT01 = pp.tile([128, L], BF16)  # normalized O^T: h0 top, h1 bottom
            OT2 = pp.tile([64, L], BF16)
            wq_sb = pp.tile([128, KCH, HCOLS], BF16)
            wk_sb = pp.tile([128, KCH, HCOLS], BF16)
            wv_sb = pp.tile([128, KCH, HCOLS], BF16)
            wT01_sb = pp.tile([128, D], BF16)
            wT2_sb = pp.tile([64, D], BF16)
            mask_i = pp.tile([128, SCH], I32)
            mask_f = pp.tile([128, SCH], F32)
            asc_bias = pp.tile([128, SCH], F32)  # ScalarE exp bias
            dve_bias = pp.tile([128, SCH], F32)  # DVE fast-exp bias
            gam_b = pp.tile([128, D], F32)
            bet_b = pp.tile([128, D], F32)
            bb_b = pp.tile([128, D], F32)

            Z_dram = dram.tile([L, D], F8)
            Zr_dram = dram.tile([NCHUNK, ORows, D], F8)
            sync_in = dram.tile([4, 192], BF16)
            sync_out = dram.tile([1, 192], BF16)

            # early dummy collective: absorbs the inter-core rendezvous
            # skew while the PE is still busy with projections, so the first
            # real ReduceScatter doesn't stall ~20us waiting for peers.
            nc.gpsimd.collective_compute(
                "ReduceScatter",
                AL.add,
                replica_groups=GROUPS,
                ins=[sync_in[:, :].opt()],
                outs=[sync_out[:, :].opt()],
            )
            # constant / weight loads
            nc.sync.dma_start(out=wq_sb, in_=wqT.rearrange("(c p) m -> p c m", p=128))
            nc.sync.dma_start(out=wk_sb, in_=wkT.rearrange("(c p) m -> p c m", p=128))
            nc.sync.dma_start(out=wv_sb, in_=wvT.rearrange("(c p) m -> p c m", p=128))
            nc.sync.dma_start(out=wT01_sb, in_=wT01[:, :])
            nc.sync.dma_start(out=wT2_sb, in_=wT2[:, :])
            nc.sync.dma_start(out=mask_i, in_=maskT[:, :])
            nc.sync.dma_start(out=gam_b, in_=gamma1.to_broadcast([128, D]))
            nc.sync.dma_start(out=bet_b, in_=beta1.to_broadcast([128, D]))
            nc.sync.dma_start(out=bb_b, in_=bias1.to_broadcast([128, D]))
            nc.vector.memset(V_sb, 0.0)
            ones_t = pp.tile([128, SCH, HPC, 1], BF16)
            nc.vector.memset(ones_t, 1.0)
            nc.vector.tensor_copy(V_sb[:, :, :, 0:1], ones_t)
            nc.vector.tensor_copy(mask_f, mask_i)
            # ScalarE: bias = (1-m)*MASK_NEG == m*(-MASK_NEG) + MASK_NEG
            nc.scalar.activation(
                asc_bias, mask_f, ACT.Copy, bias=float(MASK_NEG), scale=-MASK_NEG
            )
            # DVE: b_p = 8*mask_bias + FE_B/FE_A = m*(-8*MASK_NEG) + 8*MASK_NEG + FE_B/FE_A
            nc.scalar.activation(
                dve_bias,
                mask_f,
                ACT.Copy,
                bias=float(8.0 * MASK_NEG + FE_B_OVER_A),
                scale=-8.0 * MASK_NEG,
            )

            # PE warm-up during the initial DMA window
            warm_f = pp.tile([128, 512], F32)
            nc.vector.memset(warm_f, 0.0)
            # preload the gpsimd partition_broadcast library and the ScalarE
            # exp table set now, off the critical path: the first real calls
            # otherwise pay ~5us (LOAD_LIB) / ~2.7us (ACT_TABLE_LOAD) inside
            # the attention pipeline.
            pre_bc = pp.tile([64, 512], F32)
            nc.gpsimd.partition_broadcast(pre_bc, warm_f[0:1, :])
            pre_exp = pp.tile([128, SCH], F32)
            nc.scalar.activation(pre_exp, mask_f, ACT.Exp, bias=0.0, scale=0.0)
            warm_l = pp.tile([128, 128], BF16)
            warm_r = pp.tile([128, 512], BF16)
            nc.vector.tensor_copy(warm_l, warm_f[:, 0:128])
            nc.vector.tensor_copy(warm_r, warm_f)
            with tc.tile_pool(name="warmps", bufs=1, space="PSUM") as wps:
                for w in range(40):
                    wp = wps.tile([128, 512], F32, tag="w", bufs=2, name=f"w{w}")
                    nc.tensor.matmul(wp, warm_l, warm_r, start=True, stop=True)

            # ---- Stage A: projections (all bf16) ----
            with (
                tc.tile_pool(name="pin", bufs=1) as pin,
                tc.tile_pool(name="psp", bufs=1, space="PSUM") as psp,
            ):
                kch = []
                vch = []
                qch = []
                wwi = [0]

                def warm_touch(ch):
                    # keyed warm matmuls: depend on the arriving chunk, so the
                    # scheduler spreads PE pulses across the DMA window and the
                    # HAM clock-gate never sees a ~3.4us idle window.
                    for r in range(2):
                        wp = psp.tile(
                            [128, 512], F32, tag="pB", bufs=3, name=f"wt{wwi[0]}"
                        )
                        wwi[0] += 1
                        nc.tensor.matmul(
                            wp, warm_l, ch[:, 512 * r : 512 * (r + 1)],
                            start=True, stop=True,
                        )

                for i in range(KCH):
                    ch = pin.tile([128, S], BF16, tag="kin", bufs=KCH, name=f"kch{i}")
                    nc.sync.dma_start(out=ch, in_=kT[128 * i : 128 * (i + 1), :])
                    kch.append(ch)
                    warm_touch(ch)
                for i in range(KCH):
                    ch = pin.tile([128, S], BF16, tag="vin", bufs=KCH, name=f"vch{i}")
                    nc.sync.dma_start(out=ch, in_=vT[128 * i : 128 * (i + 1), :])
                    vch.append(ch)
                    warm_touch(ch)
                for i in range(KCH):
                    ch = pin.tile([128, L], BF16, tag="qin", bufs=KCH, name=f"qch{i}")
                    nc.sync.dma_start(out=ch, in_=qT[128 * i : 128 * (i + 1), :])
                    qch.append(ch)
                    warm_touch(ch)

                # K heads 0/1 -> KT1 (full 128 feature rows)
                for n in range(NLB):
                    nsl = slice(512 * n, 512 * (n + 1))
                    ps = psp.tile([128, 512], F32, tag="pA", bufs=3, name="psk")
                    for i in range(KCH):
                        nc.tensor.matmul(
                            ps,
                            wk_sb[:, i, 0:128],
                            kch[i][:, nsl],
                            start=(i == 0),
                            stop=(i == KCH - 1),
                        )
                    nc.vector.tensor_copy(out=KT1[:, nsl], in_=ps)

                # V projection -> V_sb[:, s, h, 0:64]
                for s in range(SCH):
                    ps = psp.tile([128, 192], F32, tag="pB", bufs=3, name="psv")
                    for i in range(KCH):
                        nc.tensor.matmul(
                            ps,
                            vch[i][:, 128 * s : 128 * (s + 1)],
                            wv_sb[:, i, :],
                            start=(i == 0),
                            stop=(i == KCH - 1),
                        )
                    nc.scalar.copy(
                        out=V_sb[:, s, :, 64:128],
                        in_=ps.rearrange("p (h d) -> p h d", h=HPC),
                    )

                # Q heads 0/1 -> QT1
                for n in range(NLB):
                    nsl = slice(512 * n, 512 * (n + 1))
                    ps = psp.tile([128, 512], F32, tag="pA", bufs=3, name="psq")
                    for i in range(KCH):
                        nc.tensor.matmul(
                            ps,
                            wq_sb[:, i, 0:128],
                            qch[i][:, nsl],
                            start=(i == 0),
                            stop=(i == KCH - 1),
                        )
                    nc.vector.tensor_copy(out=QT1[:, nsl], in_=ps)

                # head-2 Q and K projections, col-group paired (concurrent):
                # Q-m1 writes psum partitions 0:64 (col groups 0/1), K-m1 a
                # separate psum's partitions 64:128 (col groups 2/3).
                for n in range(NLB):
                    nsl = slice(512 * n, 512 * (n + 1))
                    psq2 = psp.tile([128, 512], F32, tag="pA", bufs=3, name="psq2")
                    psk2 = psp.tile([128, 512], F32, tag="pC", bufs=2, name="psk2")
                    for i in range(KCH):
                        nc.tensor.matmul(
                            psq2[0:64],
                            wq_sb[:, i, 128:192],
                            qch[i][:, nsl],
                            start=(i == 0),
                            stop=(i == KCH - 1),
                        )
                        nc.tensor.matmul(
                            psk2[64:128],
                            wk_sb[:, i, 128:192],
                            kch[i][:, nsl],
                            start=(i == 0),
                            stop=(i == KCH - 1),
                        )
                    nc.vector.tensor_copy(out=QT2[0:64, nsl], in_=psq2[0:64])
                    nc.vector.tensor_copy(out=KT2[0:64, nsl], in_=psk2[64:128])
                    nc.sync.dma_start(out=QT2[64:128, nsl], in_=QT2[0:64, nsl])
                    nc.sync.dma_start(out=KT2[64:128, nsl], in_=KT2[0:64, nsl])

                # keep-warm bridge across the pool transition
                for w in range(16):
                    wp = psp.tile([128, 512], F32, tag="pB", bufs=3, name=f"wb{w}")
                    nc.tensor.matmul(wp, warm_l, warm_r, start=True, stop=True)

            # ---- Stage B: attention + out-projection + RS + LN, per l-block ----
            def fexp_dve(dst_bf16, src_psum, scslice):
                # bf16 bits = (s + b_p) * FE_A, written as int16 (bitcast view)
                nc.vector.tensor_scalar(
                    out=dst_bf16.bitcast(I16),
                    in0=src_psum,
                    scalar1=scslice,
                    scalar2=float(FE_A),
                    op0=AL.add,
                    op1=AL.mult,
                )

            def norm_drain(o_ps, dest, lnm, drp):
                # dest <- o_ps[1:65] * (1 / denom_row) ; denom = o_ps[0:1]
                # (ones column of V is at index 0, so the denominator lands on
                # PSUM partition 0 where reciprocal_approx_fast reads correctly)
                rr = drp.tile([1, 512], F32, tag="rr", bufs=4, name=f"rr{lnm}")
                nc.vector.reciprocal_approx_fast(rr, o_ps[0:1, :])
                rb = drp.tile([64, 512], F32, tag="rb", bufs=4, name=f"rb{lnm}")
                nc.gpsimd.partition_broadcast(rb, rr)
                nc.vector.tensor_mul(dest, o_ps[64:128, :], rb)

            with (
                tc.tile_pool(name="ptp", bufs=1) as ptp,
                tc.tile_pool(name="drp", bufs=1) as drp,
                tc.tile_pool(name="zsb", bufs=6) as zsb,
                tc.tile_pool(name="aps", bufs=1, space="PSUM") as aps,
                tc.tile_pool(name="ep", bufs=2) as ep,
            ):
                def exp_tile(dst, srcp, sc, use_scalar):
                    if use_scalar:
                        nc.scalar.activation(
                            dst, srcp, ACT.Exp,
                            bias=asc_bias[:, sc : sc + 1], scale=0.125,
                        )
                    else:
                        fexp_dve(dst, srcp, dve_bias[:, sc : sc + 1])

                def ln_chunk_ops(k):
                    # LayerNorm over RS chunk k as a list of small closures so
                    # the emitter can spread them between exp tiles of the next
                    # attention phase (a monolithic LN block parks ~5us of DVE
                    # work in front of the exps and starves the PE).
                    zr = ep.tile([128, D], F8, name="zr")
                    qr = ep.tile([128, D], BF16, name="qr")
                    xb = ep.tile([128, D], F32, name="xb")
                    stats = ep.tile([128, 3, 6], F32, name="stats")
                    mv = ep.tile([128, 2], F32, name="mv")
                    ve = ep.tile([128, 1], F32, name="ve")
                    sh = ep.tile([128, 1], I32, name="sh")
                    r0i = ep.tile([128, 1], I32, name="r0i")
                    t2 = ep.tile([128, 1], F32, name="t2")
                    rstd = ep.tile([128, 1], F32, name="rstd")
                    t1 = ep.tile([128, D], F32, name="t1")
                    o = ep.tile([128, D], F32, name="o")

                    def s_dma():
                        nc.sync.dma_start(out=zr, in_=Zr_dram[k])
                        nc.sync.dma_start(out=qr, in_=qres[k])

                    def s_add():
                        nc.vector.tensor_add(xb, zr, qr)

                    def s_bn(g):
                        return lambda: nc.vector.bn_stats(
                            stats[:, g, :], xb[:, 256 * g : 256 * (g + 1)]
                        )

                    def s_aggr():
                        nc.vector.bn_aggr(mv, stats)
                        nc.vector.tensor_scalar_add(ve, mv[:, 1:2], float(LN_EPS))

                    def s_rsqrt():
                        nc.vector.tensor_scalar(
                            out=sh, in0=ve[:, 0:1].bitcast(I32), scalar1=1,
                            scalar2=None, op0=AL.arith_shift_right,
                        )
                        nc.vector.tensor_scalar(
                            out=r0i, in0=sh, scalar1=0x5F3759DF, scalar2=-1,
                            op0=AL.subtract, op1=AL.mult,
                        )
                        r0 = r0i[:, 0:1].bitcast(F32)
                        nc.vector.tensor_mul(t2, r0, r0)
                        nc.vector.tensor_mul(t2, t2, ve)
                        nc.vector.tensor_scalar(
                            out=t2, in0=t2, scalar1=-0.5, scalar2=1.5,
                            op0=AL.mult, op1=AL.add,
                        )
                        nc.vector.tensor_mul(rstd, r0, t2)

                    def s_stt1():
                        nc.vector.scalar_tensor_tensor(
                            t1, xb, mv[:, 0:1], gam_b, AL.subtract, AL.mult
                        )

                    def s_stt2():
                        nc.vector.scalar_tensor_tensor(
                            o, t1, rstd, bet_b, AL.mult, AL.add
                        )
                        nc.sync.dma_start(out=out[k], in_=o)

                    return [s_dma, s_add, s_bn(0), s_bn(1), s_bn(2),
                            s_aggr, s_rsqrt, s_stt1, s_stt2]

                def ln_chunk(k):
                    for f in ln_chunk_ops(k):
                        f()

                def z_block(lb):
                    # out-projection Z = [O0;O1]^T.W01 + O2^T.W2, RS per 2 tiles
                    for t in range(4):
                        lt = 4 * lb + t
                        tsl = slice(128 * lt, 128 * (lt + 1))
                        zp = aps.tile(
                            [128, 1024], F32, tag="sw", bufs=2, name=f"zp{lt}"
                        )
                        for n0, nw in ((0, 512), (512, 256)):
                            nsl = slice(n0, n0 + nw)
                            nc.tensor.matmul(
                                zp[:, nsl], OT01[:, tsl], wT01_sb[:, nsl],
                                start=True, stop=False,
                            )
                            nc.tensor.matmul(
                                zp[:, nsl], OT2[:, tsl], wT2_sb[:, nsl],
                                start=False, stop=True,
                            )
                        zs = zsb.tile([128, D], F8, name="zs")
                        nc.scalar.copy(out=zs, in_=zp[:, 0:768])
                        nc.sync.dma_start(out=Z_dram[tsl, :], in_=zs)
                        if t == 3:
                            nc.gpsimd.collective_compute(
                                "ReduceScatter",
                                AL.add,
                                replica_groups=GROUPS,
                                ins=[Z_dram[CROWS * lb : CROWS * (lb + 1), :].opt()],
                                outs=[Zr_dram[lb].opt()],
                            )
                    pend_ln_wait.append(lb)

                # Phase machine: h2 per lb-pair, then h0h1 per lb. Deferred
                # work (Z+RS of the previous block, LN two phases later) is
                # flushed mid-phase so it never heads the engine queues while
                # its dependencies (norm chain / collectives) are in flight.
                pend_z = []  # lb ids whose Z+RS emission is due
                pend_ln = []  # ln chunk ids due this phase
                pend_ln_wait = []  # ln chunk ids due next phase
                bg_ops = []  # deferred LN closures, drained one per sc
                h01_done = set()
                h2_done = set()
                z_queued = set()

                def flush(lst, fn):
                    while lst:
                        fn(lst.pop(0))

                def attn_phase(kind, arg):
                    if kind == "h2":
                        pair = arg
                        lb0, lb1 = 2 * pair, 2 * pair + 1
                        lsl0 = slice(512 * lb0, 512 * (lb0 + 1))
                        lsl1 = slice(512 * lb1, 512 * (lb1 + 1))
                        oC0 = aps.tile([128, 512], F32, tag="oA", bufs=2, name=f"oC{lb0}")
                        oC1 = aps.tile([128, 512], F32, tag="oB", bufs=2, name=f"oC{lb1}")
                        for sc in range(SCH):
                            ssl = slice(128 * sc, 128 * (sc + 1))
                            sw2 = aps.tile(
                                [128, 1024], F32, tag="sw", bufs=2,
                                name=f"sw2_{pair}_{sc}",
                            )
                            nc.tensor.matmul(
                                sw2[:, 0:512], KT2[0:64, ssl], QT2[0:64, lsl0],
                                start=True, stop=True,
                            )
                            nc.tensor.matmul(
                                sw2[:, 512:1024], KT2[64:128, ssl], QT2[64:128, lsl1],
                                start=True, stop=True,
                            )
                            P2 = ptp.tile([128, 1024], BF16, tag="p", bufs=4, name="P2")
                            exp_tile(P2[:, 0:1024], sw2[:, 0:1024], sc, (sc % 2) == 0)
                            nc.tensor.matmul(
                                oC0, V_sb[:, sc, 2, :], P2[:, 0:512],
                                start=(sc == 0), stop=(sc == SCH - 1),
                            )
                            nc.tensor.matmul(
                                oC1, V_sb[:, sc, 2, :], P2[:, 512:1024],
                                start=(sc == 0), stop=(sc == SCH - 1),
                            )
                            if bg_ops:
                                bg_ops.pop(0)()
                        norm_drain(oC0, OT2[:, lsl0], f"c{lb0}", drp)
                        norm_drain(oC1, OT2[:, lsl1], f"c{lb1}", drp)
                    else:
                        lb = arg
                        lsl = slice(512 * lb, 512 * (lb + 1))
                        oA = aps.tile([128, 512], F32, tag="oA", bufs=2, name=f"oA{lb}")
                        oB = aps.tile([128, 512], F32, tag="oB", bufs=2, name=f"oB{lb}")
                        for sc in range(SCH):
                            ssl = slice(128 * sc, 128 * (sc + 1))
                            sw = aps.tile(
                                [128, 1024], F32, tag="sw", bufs=2,
                                name=f"sw{lb}_{sc}",
                            )
                            nc.tensor.matmul(
                                sw[:, 0:512], KT1[0:64, ssl], QT1[0:64, lsl],
                                start=True, stop=True,
                            )
                            nc.tensor.matmul(
                                sw[:, 512:1024], KT1[64:128, ssl], QT1[64:128, lsl],
                                start=True, stop=True,
                            )
                            P = ptp.tile([128, 1024], BF16, tag="p", bufs=4, name="P")
                            exp_tile(P[:, 0:1024], sw[:, 0:1024], sc, WIDE_SCALAR[sc % 8])
                            nc.tensor.matmul(
                                oA, V_sb[:, sc, 0, :], P[:, 0:512],
                                start=(sc == 0), stop=(sc == SCH - 1),
                            )
                            nc.tensor.matmul(
                                oB, V_sb[:, sc, 1, :], P[:, 512:1024],
                                start=(sc == 0), stop=(sc == SCH - 1),
                            )
                            if bg_ops:
                                bg_ops.pop(0)()
                        norm_drain(oA, OT01[0:64, lsl], f"a{lb}", drp)
                        norm_drain(oB, OT01[64:128, lsl], f"b{lb}", drp)
                        h01_done.add(lb)
                    if kind == "h2":
                        h2_done.update((2 * arg, 2 * arg + 1))
                    # a block's Z can be emitted once both its head phases are
                    # complete; its LN two phases after the Z+RS emission.
                    for lb_ in range(NLB):
                        if lb_ in h01_done and lb_ in h2_done and lb_ not in z_queued:
                            z_queued.add(lb_)
                            pend_z.append(lb_)
                    flush(pend_z, z_block)
                    while pend_ln:
                        bg_ops.extend(ln_chunk_ops(pend_ln.pop(0)))
                    pend_ln.extend(pend_ln_wait)
                    del pend_ln_wait[:]

                attn_phase("h01", 0)
                attn_phase("h2", 0)
                attn_phase("h01", 1)
                attn_phase("h2", 1)
                attn_phase("h01", 2)
                attn_phase("h01", 3)
                # tail
                while bg_ops:
                    bg_ops.pop(0)()
                flush(pend_z, z_block)
                pend_ln.extend(pend_ln_wait)
                del pend_ln_wait[:]
                flush(pend_ln, ln_chunk)

    nc.finalize()
    return nc


def _get_nc():
    if "nc" not in _CACHE:
        _CACHE["nc"] = _build()
    return _CACHE["nc"]


def build_in_maps(inputs):
    return _build_in_maps(**inputs)


def _bf(x):
    return np.ascontiguousarray(np.asarray(x, dtype=np.float32).astype(ml_dtypes.bfloat16))


def _build_in_maps(q, k, v, attention_mask, Wq, Wk, Wv, W, b, gamma, beta):
    q = np.asarray(q, dtype=np.float32)
    k = np.asarray(k, dtype=np.float32)
    v = np.asarray(v, dtype=np.float32)
    attention_mask = np.asarray(attention_mask, dtype=np.int32)
    Wq = np.asarray(Wq, dtype=np.float32)
    Wk = np.asarray(Wk, dtype=np.float32)
    Wv = np.asarray(Wv, dtype=np.float32)
    W = np.asarray(W, dtype=np.float32)
    b = np.asarray(b, dtype=np.float32)
    gamma = np.asarray(gamma, dtype=np.float32)
    beta = np.asarray(beta, dtype=np.float32)

    qT = [_bf(q[i].T) for i in range(B)]
    kT = [_bf(k[i].T) for i in range(B)]
    vT = [_bf(v[i].T) for i in range(B)]
    maskT = [
        np.ascontiguousarray(attention_mask[i].reshape(SCH, 128).T) for i in range(B)
    ]
    bias1 = np.ascontiguousarray(b.reshape(1, D))
    gamma1 = np.ascontiguousarray(gamma.reshape(1, D))
    beta1 = np.ascontiguousarray(beta.reshape(1, D))

    in_maps = []
    for c in range(NCORES):
        bi, hg = c // 4, c % 4
        cs = slice(HCOLS * hg, HCOLS * (hg + 1))
        wT = W[:, cs].T  # [192, 768]
        in_maps.append(
            {
                "qT": qT[bi],
                "kT": kT[bi],
                "vT": vT[bi],
                "wqT": _bf(Wq[cs, :].T),
                "wkT": _bf(Wk[cs, :].T),
                "wvT": _bf(Wv[cs, :].T),
                "wT01": _bf(wT[0:128]),
                "wT2": _bf(wT[128:192]),
                "qres": _bf(
                    np.stack(
                        [
                            q[
                                bi,
                                CROWS * j + ORows * hg : CROWS * j + ORows * (hg + 1),
                                :,
                            ]
                            + b[None, :]
                            for j in range(NCHUNK)
                        ]
                    )
                ),
                "maskT": maskT[bi],
                "bias1": bias1,
                "gamma1": gamma1,
                "beta1": beta1,
            }
        )
    return in_maps


def kernel(q, k, v, attention_mask, Wq, Wk, Wv, W, b, gamma, beta):
    nc = _get_nc()
    in_maps = _build_in_maps(q, k, v, attention_mask, Wq, Wk, Wv, W, b, gamma, beta)
    res = run_bass_kernel_spmd(nc, in_maps, core_ids=list(range(NCORES)))

    outp = np.empty((B, L, D), dtype=np.float32)
    for c in range(NCORES):
        bi, hg = c // 4, c % 4
        o = res.results[c]["out"]
        for j in range(NCHUNK):
            outp[bi, CROWS * j + ORows * hg : CROWS * j + ORows * (hg + 1), :] = o[j]
    return outp
